# revision 1
# baseline (speedup 1.0000x reference)
"""Bidirectional LSTM (B=64, T=512, I=512, H=1024) on Trainium2.

Strategy: the recurrence is serial in time and the per-step GEMM
(h @ w_hh.T, contraction over H) cannot be split across cores without
per-step cross-core traffic (collective latency floor ~5us >> step
budget), so each direction runs entirely on one NeuronCore:
core 0 = forward, core 1 = backward (backward sees time-reversed x,
matching the reference which concatenates un-re-reversed).

Per core, per timestep (all matmuls bf16 with fp32 PSUM accumulation):
  gates[b, :] = [x_t | h_{t-1}] @ [w_ih | w_hh].T + b  via 12 K-tiles of 128,
  with the batch (64) as the stationary operand and the weights streamed as
  the moving operand.  Two column-tile groups (tile_position (0,0)/(0,64))
  run concurrently, each producing one half of the 4096 gate columns, so
  PSUM partitions are p = b + 64*s.  Bias is added by a K=2 matmul against
  a constant [2,128] selector.  Gate columns are host-permuted so each
  512-wide PSUM chunk is [i|f|o|g] x 128 for one 128-slice of H, keeping
  every elementwise op partition-aligned: sigmoid/tanh on ScalarE straight
  from PSUM, c-state kept fp32 in SBUF, h produced bf16 and transposed
  back into stationary layout with an SBUF->SBUF DMA transpose.
"""

import os
import sys
import numpy as np
import ml_dtypes

sys.path.insert(0, "/opt/trn_rl_repo")

import concourse.bass as bass  # noqa: E402
import concourse.bacc as bacc  # noqa: E402
import concourse.tile as tile  # noqa: E402
from concourse import mybir  # noqa: E402
from concourse.bass_utils import run_bass_kernel_spmd  # noqa: E402

BF = mybir.dt.bfloat16
F32 = mybir.dt.float32
AF = mybir.ActivationFunctionType
bf = ml_dtypes.bfloat16

B, T, I, H = 64, 512, 512, 1024
GMAP = [0, 1, 3, 2]  # chunk sub-block order (i, f, o, g) -> pytorch gate block


def _host_prep(x, w_ih, w_hh, b_ih, b_hh, reverse_time):
    if reverse_time:
        x = x[:, ::-1, :]
    # xT[t, i', q, b] = x[b, t, 128q + i']   (bf16, stationary-ready layout)
    xb = np.ascontiguousarray(x).astype(bf)
    arrI = xb.transpose(2, 1, 0)  # [I, T, B]
    xT = np.ascontiguousarray(arrI.reshape(4, 128, T, B).transpose(2, 1, 0, 3))

    W_cat = np.concatenate([w_ih, w_hh], axis=1)  # [4096, 1536]
    rows = np.zeros(4096, np.int64)
    n = 0
    for s in range(2):
        for c in range(4):
            for beta in range(4):
                base = GMAP[beta] * 1024 + 512 * s + 128 * c
                rows[n : n + 128] = np.arange(base, base + 128)
                n += 128
    Wp = W_cat[rows, :]
    wmov = np.ascontiguousarray(Wp.T.reshape(12, 128, 4096)).astype(bf)

    bperm = (b_ih + b_hh).astype(np.float32)[rows]
    bmov = np.ascontiguousarray(bperm.reshape(2, 2048)).astype(bf)

    ones2 = np.zeros((2, 128), bf)
    ones2[0, 0:64] = 1
    ones2[1, 64:128] = 1
    return {"xT": xT, "wmov": wmov, "bmov": bmov, "ones2": ones2}


def _host_post(hs, c_out):
    hsf = np.asarray(hs).astype(np.float32).reshape(T, 2, 64, 512)
    hs_full = hsf.transpose(0, 2, 1, 3).reshape(T, 64, 1024)  # [T, B, H]
    cf = np.asarray(c_out).astype(np.float32).reshape(2, 64, 512)
    c_T = cf.transpose(1, 0, 2).reshape(64, 1024)
    return hs_full, hs_full[-1].copy(), c_T


def _build_nc(psum_bufs=4):
    nc = bacc.Bacc("TRN2", target_bir_lowering=False, debug=False)
    xT_d = nc.dram_tensor("xT", [T, 128, 4 * 64], BF, kind="ExternalInput")
    wmov_d = nc.dram_tensor("wmov", [12, 128, 4096], BF, kind="ExternalInput")
    bmov_d = nc.dram_tensor("bmov", [2, 2048], BF, kind="ExternalInput")
    ones2_d = nc.dram_tensor("ones2", [2, 128], BF, kind="ExternalInput")
    hs_d = nc.dram_tensor("hs", [T, 128, 512], BF, kind="ExternalOutput")
    c_out_d = nc.dram_tensor("c_out", [128, 512], F32, kind="ExternalOutput")

    import contextlib

    with tile.TileContext(nc) as tc, contextlib.ExitStack() as ctx:
        wpool = ctx.enter_context(tc.tile_pool(name="w", bufs=1))
        cpool = ctx.enter_context(tc.tile_pool(name="const", bufs=1))
        xpool = ctx.enter_context(tc.tile_pool(name="x", bufs=3))
        hTpool = ctx.enter_context(tc.tile_pool(name="hT", bufs=2))
        hpool = ctx.enter_context(tc.tile_pool(name="h", bufs=2))
        spool = ctx.enter_context(tc.tile_pool(name="s", bufs=3))
        pspool = ctx.enter_context(
            tc.tile_pool(name="ps", bufs=psum_bufs, space="PSUM")
        )

        wm = wpool.tile([128, 12 * 4096], BF, tag="wm")
        for kt in range(12):
            nc.sync.dma_start(wm[:, 4096 * kt : 4096 * (kt + 1)], wmov_d.ap()[kt])
        bm = cpool.tile([2, 2048], BF, tag="bm")
        nc.sync.dma_start(bm[:], bmov_d.ap()[:])
        ones_t = cpool.tile([2, 128], BF, tag="ones")
        nc.sync.dma_start(ones_t[:], ones2_d.ap()[:])
        c_state = cpool.tile([128, 512], F32, tag="c")
        nc.gpsimd.memset(c_state[:], 0.0)

        hT_prev = hTpool.tile([128, 512], BF, tag="hT")
        nc.gpsimd.memset(hT_prev[:], 0.0)

        for t in range(T):
            xt = xpool.tile([128, 256], BF, tag="xt")
            nc.sync.dma_start(xt[:], xT_d.ap()[t])
            hT_next = hTpool.tile([128, 512], BF, tag="hT")
            h_sb = hpool.tile([128, 512], BF, tag="h")
            for c in range(4):
                ps = pspool.tile([128, 512], F32, tag="ps")
                nc.tensor.matmul(
                    ps[:],
                    ones_t[:],
                    bm[:, 512 * c : 512 * (c + 1)],
                    start=True,
                    stop=False,
                    skip_group_check=True,
                )
                ktlist = [("x", q) for q in range(4)] + [
                    ("h", k) for k in (0, 4, 1, 5, 2, 6, 3, 7)
                ]
                for pos, (kind, k) in enumerate(ktlist):
                    last = pos == len(ktlist) - 1
                    wslab = k if kind == "x" else 4 + k
                    for s in (0, 1):
                        if kind == "x":
                            lhsT = xt[:, 64 * k : 64 * k + 64]
                        else:
                            off = 128 * (k % 4) + 64 * (k // 4)
                            lhsT = hT_prev[:, off : off + 64]
                        base = 4096 * wslab + 2048 * s + 512 * c
                        nc.tensor.matmul(
                            ps[64 * s : 64 * s + 64, :],
                            lhsT,
                            wm[:, base : base + 512],
                            start=False,
                            stop=last and s == 1,
                            tile_position=(0, 64 * s),
                            skip_group_check=True,
                        )
                # chunk cols: 0:128 i, 128:256 f, 256:384 o, 384:512 g
                sig = spool.tile([128, 384], F32, tag="sig")
                nc.scalar.activation(sig[:], ps[:, 0:384], AF.Sigmoid)
                tng = spool.tile([128, 128], F32, tag="tng")
                nc.scalar.activation(tng[:], ps[:, 384:512], AF.Tanh)
                t1 = spool.tile([128, 128], F32, tag="t1")
                nc.vector.tensor_mul(t1[:], sig[:, 0:128], tng[:])
                t2 = spool.tile([128, 128], F32, tag="t2")
                nc.vector.tensor_mul(
                    t2[:], sig[:, 128:256], c_state[:, 128 * c : 128 * (c + 1)]
                )
                nc.vector.tensor_add(
                    c_state[:, 128 * c : 128 * (c + 1)], t1[:], t2[:]
                )
                thc = spool.tile([128, 128], F32, tag="thc")
                nc.scalar.activation(
                    thc[:], c_state[:, 128 * c : 128 * (c + 1)], AF.Tanh
                )
                nc.vector.tensor_mul(
                    h_sb[:, 128 * c : 128 * (c + 1)], sig[:, 256:384], thc[:]
                )
                nc.sync.dma_start(
                    hT_next[:, 128 * c : 128 * (c + 1)],
                    h_sb[:, 128 * c : 128 * (c + 1)],
                    transpose=True,
                )
            nc.sync.dma_start(hs_d.ap()[t], h_sb[:])
            hT_prev = hT_next
        nc.sync.dma_start(c_out_d.ap()[:], c_state[:])
    nc.compile()
    return nc


_NC_CACHE = None


def _get_nc():
    global _NC_CACHE
    if _NC_CACHE is None:
        _NC_CACHE = _build_nc()
    return _NC_CACHE


def kernel(
    x,
    w_ih_fw,
    w_hh_fw,
    b_ih_fw,
    b_hh_fw,
    w_ih_bw,
    w_hh_bw,
    b_ih_bw,
    b_hh_bw,
):
    x = np.asarray(x, np.float32)
    nc = _get_nc()
    ins_fw = _host_prep(x, w_ih_fw, w_hh_fw, b_ih_fw, b_hh_fw, reverse_time=False)
    ins_bw = _host_prep(x, w_ih_bw, w_hh_bw, b_ih_bw, b_hh_bw, reverse_time=True)
    res = run_bass_kernel_spmd(nc, [ins_fw, ins_bw], core_ids=[0, 1])
    hs_fw, h_fw, c_fw = _host_post(res.results[0]["hs"], res.results[0]["c_out"])
    hs_bw, h_bw, c_bw = _host_post(res.results[1]["hs"], res.results[1]["c_out"])
    outputs = np.concatenate([hs_fw, hs_bw], axis=-1).transpose(1, 0, 2)
    return (
        np.ascontiguousarray(outputs, dtype=np.float32),
        h_fw.astype(np.float32),
        c_fw.astype(np.float32),
        h_bw.astype(np.float32),
        c_bw.astype(np.float32),
    )
